# revision 6
# baseline (speedup 1.0000x reference)
"""Trainium2 Bass kernel for DeformablePatchSampler2d.

out[n, m, c, i, j] = bilinear_sample(x[n, c], row=RY[m, j], col=CX[m, i])

The sampling grid is batch/channel-invariant and known on the host from
`offset`, so all windows/weights are baked in at build time. Bilinear is
separable: stage 1 interpolates image rows (taps over j), stage 2 image
cols (taps over i).

v2 structure (per core, data-parallel over batch N=8):
  - 8 row-band DMA loads [128=(qh half, c), 20 rows x 126 cols] with both
    column-half origins folded into one AP (big segments, few descriptors)
  - per-patch 18x18 window gathers as ScalarE copies (no DMA descriptors)
  - stacks = patch pairs (mh, mw)+(mh+4, mw) -> both halves of the same
    band tile; f32 tensor_tensor on VectorE/GpSimdE with zero-stride
    broadcast weight APs
  - outputs merged 4 stacks per store [128, 4x256] -> one DMA per band
"""
import numpy as np

_P = 16
_NPH = _NPW = 8
_M = 64
_H = _W = 384
_C = 64
_N = 8
_QW = 126          # cols per column-half band
_RW = 20           # rows per row-band
_WPS = 96          # weight floats per stack: 2 stages * 3 taps * 16
_GPS_FRAC = 0.43   # fraction of stacks on gpsimd


def _precompute(offset: np.ndarray):
    """Window origins + 3-tap weights, f32 coord math mirroring the reference."""
    offset = offset.astype(np.float32)
    one, half = np.float32(1.0), np.float32(0.5)
    ch = np.linspace(0.0, float(_H), _NPH + 4).astype(np.float32)[2:-2]
    cw = np.linspace(0.0, float(_W), _NPW + 4).astype(np.float32)[2:-2]
    rel = np.arange(_P, dtype=np.float32) - np.float32(_P // 2)
    a = np.arange(_M) // _NPW
    b = np.arange(_M) % _NPW
    hc = ch[a][:, None] + rel[None, :]
    wcen = cw[b][:, None] + rel[None, :]
    gx = (np.float32(2.0) * hc / np.float32(_H - 1) - one) + offset[:, 0:1]
    gy = (np.float32(2.0) * wcen / np.float32(_W - 1) - one) + offset[:, 1:2]
    CX = (((gx + one) * np.float32(_W) - one) * half).astype(np.float64)  # (M,16) cols, dim i
    RY = (((gy + one) * np.float32(_H) - one) * half).astype(np.float64)  # (M,16) rows, dim j

    r0 = np.floor(RY[:, 0]).astype(np.int64)
    c0 = np.floor(CX[:, 0]).astype(np.int64)
    t_r = RY - (r0[:, None] + np.arange(_P)[None, :])
    t_c = CX - (c0[:, None] + np.arange(_P)[None, :])
    assert (t_r >= 0).all() and (t_r < 2).all()
    assert (t_c >= 0).all() and (t_c < 2).all()
    assert r0.min() >= 0 and (r0 + 17).max() <= _H - 1
    assert c0.min() >= 0 and (c0 + 17).max() <= _W - 1

    def taps(t):
        w0 = np.maximum(0.0, 1.0 - t)
        w2 = np.maximum(0.0, t - 1.0)
        return np.stack([w0, 1.0 - w0 - w2, w2], axis=-1).astype(np.float32)

    wr = taps(t_r)  # (M, 16, 3) applies to j (rows)
    wc = taps(t_c)  # (M, 16, 3) applies to i (cols)
    nt_r = np.where(np.abs(wr[:, :, 2]).max(axis=1) > 0, 3, 2)
    nt_c = np.where(np.abs(wc[:, :, 2]).max(axis=1) > 0, 3, 2)
    return r0, c0, wr, wc, nt_r, nt_c


def _plan(offset: np.ndarray):
    """Band layout, stack pairing, weight packing."""
    r0, c0, wr, wc, nt_r, nt_c = _precompute(offset)
    mh_of = np.arange(_M) // _NPW
    mw_of = np.arange(_M) % _NPW

    # row bands: one per mw; rows [band_r0, band_r0 + 20)
    band_r0 = np.array([r0[mw_of == mw].min() for mw in range(_NPW)])
    assert all((r0[m] + 17) - band_r0[mw_of[m]] <= _RW - 1 for m in range(_M))
    # column halves: qh0 = mh 0..3, qh1 = mh 4..7
    qh_start = np.array([c0[mh_of <= 3].min(), c0[mh_of >= 4].min()])
    assert (c0[mh_of <= 3] + 17 - qh_start[0]).max() <= _QW - 1
    assert (c0[mh_of >= 4] + 17 - qh_start[1]).max() <= _QW - 1
    assert qh_start[0] >= 0 and qh_start[1] + _QW <= _W

    w_all = np.zeros((128, 32 * _WPS), dtype=np.float32)
    bands = []
    for mw in range(_NPW):
        stacks = []
        for mh in range(4):
            ms = [mh * _NPW + mw, (mh + 4) * _NPW + mw]
            k = mw * 4 + mh
            rt = int(max(nt_r[ms[0]], nt_r[ms[1]]))
            ct = int(max(nt_c[ms[0]], nt_c[ms[1]]))
            for s, m in enumerate(ms):
                rows = slice(s * 64, (s + 1) * 64)
                base = k * _WPS
                for jk in range(3):
                    w_all[rows, base + jk * 16:base + jk * 16 + 16] = wr[m, :, jk][None, :]
                for ik in range(3):
                    w_all[rows, base + 48 + ik * 16:base + 48 + ik * 16 + 16] = wc[m, :, ik][None, :]
            stacks.append(dict(
                k=k, ms=ms, rt=rt, ct=ct,
                r_off=[int(r0[m] - band_r0[mw]) for m in ms],
                c_off=[int(c0[ms[s]] - qh_start[s]) for s in range(2)]))
        bands.append(dict(mw=mw, r0=int(band_r0[mw]), stacks=stacks))
    return bands, int(qh_start[0]), int(qh_start[1]), w_all


def _build(bands, qh0, qh1):
    import concourse.bacc as bacc
    import concourse.mybir as mybir
    from concourse.bass import AP
    from concourse.tile import TileContext

    f32 = mybir.dt.float32
    mult = mybir.AluOpType.mult
    add = mybir.AluOpType.add

    nc = bacc.Bacc("TRN2", target_bir_lowering=False)
    x_n = nc.dram_tensor("x_n", (_C, _H, _W), f32, kind="ExternalInput")
    w_d = nc.dram_tensor("w_all", (128, 32 * _WPS), f32, kind="ExternalInput")
    out_n = nc.dram_tensor("out_n", (_M, _C, _P, _P), f32, kind="ExternalOutput")

    def sub_ap(base_ap, extra_off, free_dims):
        return AP(base_ap.tensor, base_ap.offset + extra_off,
                  [list(base_ap.ap[0])] + [list(d) for d in free_dims])

    # which stacks run on gpsimd (spread evenly)
    n_gps = int(round(32 * _GPS_FRAC))
    gps_set = set(np.linspace(0, 31, n_gps, dtype=int).tolist())

    with TileContext(nc) as tc:
        with tc.tile_pool(name="wpool", bufs=1) as wpool, \
             tc.tile_pool(name="bpool", bufs=2) as bpool, \
             tc.tile_pool(name="apool", bufs=6) as apool, \
             tc.tile_pool(name="tpool", bufs=4) as tpool, \
             tc.tile_pool(name="mpool", bufs=6) as mpool, \
             tc.tile_pool(name="opool", bufs=2) as opool:
            W_sb = wpool.tile([128, 32 * _WPS], f32)
            nc.sync.dma_start(out=W_sb[:], in_=w_d[:])
            wb = W_sb[:]
            for band in bands:
                mw, br0 = band["mw"], band["r0"]
                B = bpool.tile([128, _RW * _QW], f32)
                # two DMAs, one per column half (disjoint partition halves ->
                # disjoint SDMA engine sets, so they overlap)
                for s, qs in enumerate((qh0, qh1)):
                    src = AP(x_n[:].tensor, br0 * _W + qs,
                             [[_H * _W, _C], [_W, _RW], [1, _QW]])
                    nc.sync.dma_start(out=B[s * 64:(s + 1) * 64, :], in_=src)
                bb = B[:]
                O4 = opool.tile([128, 4 * 256], f32)
                ob4 = O4[:]
                for st in band["stacks"]:
                    k = st["k"]
                    mh = k % 4
                    eng = nc.gpsimd if k in gps_set else nc.vector
                    # gather 18x18 windows into the A stack (ScalarE copies)
                    A = apool.tile([128, 324], f32)
                    ab = A[:]
                    for s in range(2):
                        src_w = AP(bb.tensor,
                                   bb.offset + s * 64 * (_RW * _QW)
                                   + st["r_off"][s] * _QW + st["c_off"][s],
                                   [[_RW * _QW, 64], [_QW, 18], [1, 18]])
                        dst_w = AP(ab.tensor, ab.offset + s * 64 * 324,
                                   [[324, 64], [18, 18], [1, 18]])
                        nc.scalar.copy(out=dst_w, in_=src_w)
                    base = k * _WPS
                    # stage 1: T[p, (j, q)] = sum_jk wr[jk, j] * A[p, (j+jk, q)]
                    T = tpool.tile([128, 288], f32)
                    tb = T[:]
                    t_ap = sub_ap(tb, 0, [[18, 16], [1, 18]])
                    for jk in range(st["rt"]):
                        a_ap = sub_ap(ab, jk * 18, [[18, 16], [1, 18]])
                        w_ap = sub_ap(wb, base + jk * 16, [[1, 16], [0, 18]])
                        if jk == 0:
                            eng.tensor_tensor(out=t_ap, in0=a_ap, in1=w_ap, op=mult)
                        else:
                            Mt = mpool.tile([128, 288], f32)
                            m_ap = sub_ap(Mt[:], 0, [[18, 16], [1, 18]])
                            eng.tensor_tensor(out=m_ap, in0=a_ap, in1=w_ap, op=mult)
                            eng.tensor_tensor(out=t_ap, in0=t_ap, in1=m_ap, op=add)
                    # stage 2: O[p, (i, j)] = sum_ik wc[ik, i] * T[p, (j, i+ik)]
                    o_ap = sub_ap(ob4, mh * 256, [[16, 16], [1, 16]])
                    for ik in range(st["ct"]):
                        t2_ap = sub_ap(tb, ik, [[1, 16], [18, 16]])
                        w_ap = sub_ap(wb, base + 48 + ik * 16, [[1, 16], [0, 16]])
                        if ik == 0:
                            eng.tensor_tensor(out=o_ap, in0=t2_ap, in1=w_ap, op=mult)
                        else:
                            Mt = mpool.tile([128, 288], f32)
                            m_ap = sub_ap(Mt[:], 0, [[16, 16], [1, 16]])
                            eng.tensor_tensor(out=m_ap, in0=t2_ap, in1=w_ap, op=mult)
                            eng.tensor_tensor(out=o_ap, in0=o_ap, in1=m_ap, op=add)
                # stores: m = mh*8 + mw + s*32, free (mh, i*16+j); one per half
                for s in range(2):
                    dst = AP(out_n[:].tensor, (s * 32 + mw) * (_C * 256),
                             [[256, _C], [8 * _C * 256, 4], [1, 256]])
                    nc.sync.dma_start(out=dst, in_=O4[s * 64:(s + 1) * 64, :])
    nc.compile()
    return nc


def _run(nc, x, w_all, **kwargs):
    from concourse.bass_utils import run_bass_kernel_spmd
    in_maps = [{"x_n": np.ascontiguousarray(x[n]), "w_all": w_all}
               for n in range(_N)]
    return run_bass_kernel_spmd(nc, in_maps, core_ids=list(range(_N)), **kwargs)


def _prepare(offset):
    bands, qh0, qh1, w_all = _plan(offset)
    nc = _build(bands, qh0, qh1)
    return nc, w_all


def kernel(x: np.ndarray, offset: np.ndarray) -> np.ndarray:
    x = np.asarray(x, dtype=np.float32)
    offset = np.asarray(offset, dtype=np.float32)
    nc, w_all = _prepare(offset)
    res = _run(nc, x, w_all)
    return np.stack([res.results[n]["out_n"] for n in range(_N)])


# revision 9
# speedup vs baseline: 1.1585x; 1.1585x over previous
"""Trainium2 Bass kernel for DeformablePatchSampler2d.

out[n, m, c, i, j] = bilinear_sample(x[n, c], row=RY[m, j], col=CX[m, i])

The sampling grid is batch/channel-invariant and known on the host from
`offset`, so all windows/weights are baked in at build time. Bilinear is
separable: stage 1 interpolates image rows (taps over j), stage 2 image
cols (taps over i).

v2 structure (per core, data-parallel over batch N=8):
  - 8 row-band DMA loads [128=(qh half, c), 20 rows x 126 cols] with both
    column-half origins folded into one AP (big segments, few descriptors)
  - per-patch 18x18 window gathers as ScalarE copies (no DMA descriptors)
  - stacks = patch pairs (mh, mw)+(mh+4, mw) -> both halves of the same
    band tile; f32 tensor_tensor on VectorE/GpSimdE with zero-stride
    broadcast weight APs
  - outputs merged 4 stacks per store [128, 4x256] -> one DMA per band
"""
import numpy as np

_P = 16
_NPH = _NPW = 8
_M = 64
_H = _W = 384
_C = 64
_N = 8
_QW = 126          # cols per column-half band
_RW = 20           # rows per row-band
_WPS = 96          # weight floats per stack: 2 stages * 3 taps * 16
_GPS_FRAC = 0.35   # fraction of stacks on gpsimd


def _precompute(offset: np.ndarray):
    """Window origins + 3-tap weights, f32 coord math mirroring the reference."""
    offset = offset.astype(np.float32)
    one, half = np.float32(1.0), np.float32(0.5)
    ch = np.linspace(0.0, float(_H), _NPH + 4).astype(np.float32)[2:-2]
    cw = np.linspace(0.0, float(_W), _NPW + 4).astype(np.float32)[2:-2]
    rel = np.arange(_P, dtype=np.float32) - np.float32(_P // 2)
    a = np.arange(_M) // _NPW
    b = np.arange(_M) % _NPW
    hc = ch[a][:, None] + rel[None, :]
    wcen = cw[b][:, None] + rel[None, :]
    gx = (np.float32(2.0) * hc / np.float32(_H - 1) - one) + offset[:, 0:1]
    gy = (np.float32(2.0) * wcen / np.float32(_W - 1) - one) + offset[:, 1:2]
    CX = (((gx + one) * np.float32(_W) - one) * half).astype(np.float64)  # (M,16) cols, dim i
    RY = (((gy + one) * np.float32(_H) - one) * half).astype(np.float64)  # (M,16) rows, dim j

    r0 = np.floor(RY[:, 0]).astype(np.int64)
    c0 = np.floor(CX[:, 0]).astype(np.int64)
    t_r = RY - (r0[:, None] + np.arange(_P)[None, :])
    t_c = CX - (c0[:, None] + np.arange(_P)[None, :])
    assert (t_r >= 0).all() and (t_r < 2).all()
    assert (t_c >= 0).all() and (t_c < 2).all()
    assert r0.min() >= 0 and (r0 + 17).max() <= _H - 1
    assert c0.min() >= 0 and (c0 + 17).max() <= _W - 1

    def taps(t):
        w0 = np.maximum(0.0, 1.0 - t)
        w2 = np.maximum(0.0, t - 1.0)
        return np.stack([w0, 1.0 - w0 - w2, w2], axis=-1).astype(np.float32)

    wr = taps(t_r)  # (M, 16, 3) applies to j (rows)
    wc = taps(t_c)  # (M, 16, 3) applies to i (cols)
    nt_r = np.where(np.abs(wr[:, :, 2]).max(axis=1) > 0, 3, 2)
    nt_c = np.where(np.abs(wc[:, :, 2]).max(axis=1) > 0, 3, 2)
    return r0, c0, wr, wc, nt_r, nt_c


def _plan(offset: np.ndarray):
    """Band layout, stack pairing, weight packing."""
    r0, c0, wr, wc, nt_r, nt_c = _precompute(offset)
    mh_of = np.arange(_M) // _NPW
    mw_of = np.arange(_M) % _NPW

    # row bands: one per mw; rows [band_r0, band_r0 + 20)
    band_r0 = np.array([r0[mw_of == mw].min() for mw in range(_NPW)])
    assert all((r0[m] + 17) - band_r0[mw_of[m]] <= _RW - 1 for m in range(_M))
    # column halves: qh0 = mh 0..3, qh1 = mh 4..7
    qh_start = np.array([c0[mh_of <= 3].min(), c0[mh_of >= 4].min()])
    assert (c0[mh_of <= 3] + 17 - qh_start[0]).max() <= _QW - 1
    assert (c0[mh_of >= 4] + 17 - qh_start[1]).max() <= _QW - 1
    assert qh_start[0] >= 0 and qh_start[1] + _QW <= _W

    w_all = np.zeros((128, 32 * _WPS), dtype=np.float32)
    bands = []
    for mw in range(_NPW):
        stacks = []
        for mh in range(4):
            ms = [mh * _NPW + mw, (mh + 4) * _NPW + mw]
            k = mw * 4 + mh
            rt = int(max(nt_r[ms[0]], nt_r[ms[1]]))
            ct = int(max(nt_c[ms[0]], nt_c[ms[1]]))
            for s, m in enumerate(ms):
                rows = slice(s * 64, (s + 1) * 64)
                base = k * _WPS
                for jk in range(3):
                    w_all[rows, base + jk * 16:base + jk * 16 + 16] = wr[m, :, jk][None, :]
                for ik in range(3):
                    w_all[rows, base + 48 + ik * 16:base + 48 + ik * 16 + 16] = wc[m, :, ik][None, :]
            stacks.append(dict(
                k=k, ms=ms, rt=rt, ct=ct,
                r_off=[int(r0[m] - band_r0[mw]) for m in ms],
                c_off=[int(c0[ms[s]] - qh_start[s]) for s in range(2)]))
        bands.append(dict(mw=mw, r0=int(band_r0[mw]), stacks=stacks))
    return bands, int(qh_start[0]), int(qh_start[1]), w_all


def _build(bands, qh0, qh1):
    import concourse.bacc as bacc
    import concourse.mybir as mybir
    from concourse.bass import AP
    from concourse.tile import TileContext

    f32 = mybir.dt.float32
    mult = mybir.AluOpType.mult
    add = mybir.AluOpType.add

    nc = bacc.Bacc("TRN2", target_bir_lowering=False)
    x_n = nc.dram_tensor("x_n", (_C, _H, _W), f32, kind="ExternalInput")
    w_d = nc.dram_tensor("w_all", (128, 32 * _WPS), f32, kind="ExternalInput")
    out_n = nc.dram_tensor("out_n", (_M, _C, _P, _P), f32, kind="ExternalOutput")

    def sub_ap(base_ap, extra_off, free_dims):
        return AP(base_ap.tensor, base_ap.offset + extra_off,
                  [list(base_ap.ap[0])] + [list(d) for d in free_dims])

    # which stacks run on gpsimd (spread evenly)
    n_gps = int(round(32 * _GPS_FRAC))
    gps_set = set(np.linspace(0, 31, n_gps, dtype=int).tolist())

    with TileContext(nc) as tc:
        with tc.tile_pool(name="wpool", bufs=1) as wpool, \
             tc.tile_pool(name="wpsp", bufs=1, space="PSUM") as wpsp, \
             tc.tile_pool(name="bpool", bufs=3) as bpool, \
             tc.tile_pool(name="apool", bufs=8) as apool, \
             tc.tile_pool(name="tpool", bufs=6) as tpool, \
             tc.tile_pool(name="mpool", bufs=6) as mpool, \
             tc.tile_pool(name="mpsp", bufs=2, space="PSUM") as mpsp, \
             tc.tile_pool(name="opool", bufs=3) as opool:
            W_sb = wpool.tile([128, 32 * _WPS], f32)
            nc.sync.dma_start(out=W_sb[:], in_=w_d[:])
            # PSUM copy of the weights: DVE ops read weights/temps via the
            # dedicated PSUM port, leaving the shared SBUF port pair to
            # GpSimd so both engines run truly concurrently.
            W_ps = wpsp.tile([128, 32 * _WPS], f32)
            nc.scalar.copy(out=W_ps[:], in_=W_sb[:])
            wb_s, wb_p = W_sb[:], W_ps[:]
            for band in bands:
                mw, br0 = band["mw"], band["r0"]
                B = bpool.tile([128, _RW * _QW], f32)
                # two DMAs, one per column half (disjoint partition halves ->
                # disjoint SDMA engine sets, so they overlap)
                for s, qs in enumerate((qh0, qh1)):
                    src = AP(x_n[:].tensor, br0 * _W + qs,
                             [[_H * _W, _C], [_W, _RW], [1, _QW]])
                    nc.sync.dma_start(out=B[s * 64:(s + 1) * 64, :], in_=src)
                bb = B[:]
                O4 = opool.tile([128, 4 * 256], f32)
                ob4 = O4[:]
                for st in band["stacks"]:
                    k = st["k"]
                    mh = k % 4
                    on_gps = k in gps_set
                    eng = nc.gpsimd if on_gps else nc.vector
                    wb = wb_s if on_gps else wb_p
                    mp = mpool if on_gps else mpsp
                    # gather 18x18 windows into the A stack (ScalarE copies)
                    A = apool.tile([128, 324], f32)
                    ab = A[:]
                    for s in range(2):
                        src_w = AP(bb.tensor,
                                   bb.offset + s * 64 * (_RW * _QW)
                                   + st["r_off"][s] * _QW + st["c_off"][s],
                                   [[_RW * _QW, 64], [_QW, 18], [1, 18]])
                        dst_w = AP(ab.tensor, ab.offset + s * 64 * 324,
                                   [[324, 64], [18, 18], [1, 18]])
                        nc.scalar.copy(out=dst_w, in_=src_w)
                    base = k * _WPS
                    qn = 16 + st["ct"]  # stage-2 only reads q in [0, 16+ct)
                    # stage 1: T[p, (j, q)] = sum_jk wr[jk, j] * A[p, (j+jk, q)]
                    T = tpool.tile([128, 288], f32)
                    tb = T[:]
                    t_ap = sub_ap(tb, 0, [[18, 16], [1, qn]])
                    for jk in range(st["rt"]):
                        a_ap = sub_ap(ab, jk * 18, [[18, 16], [1, qn]])
                        w_ap = sub_ap(wb, base + jk * 16, [[1, 16], [0, qn]])
                        if jk == 0:
                            eng.tensor_tensor(out=t_ap, in0=a_ap, in1=w_ap, op=mult)
                        else:
                            Mt = mp.tile([128, 288], f32)
                            m_ap = sub_ap(Mt[:], 0, [[18, 16], [1, qn]])
                            eng.tensor_tensor(out=m_ap, in0=a_ap, in1=w_ap, op=mult)
                            eng.tensor_tensor(out=t_ap, in0=t_ap, in1=m_ap, op=add)
                    # stage 2: O[p, (i, j)] = sum_ik wc[ik, i] * T[p, (j, i+ik)]
                    o_ap = sub_ap(ob4, mh * 256, [[16, 16], [1, 16]])
                    for ik in range(st["ct"]):
                        t2_ap = sub_ap(tb, ik, [[1, 16], [18, 16]])
                        w_ap = sub_ap(wb, base + 48 + ik * 16, [[1, 16], [0, 16]])
                        if ik == 0:
                            eng.tensor_tensor(out=o_ap, in0=t2_ap, in1=w_ap, op=mult)
                        else:
                            Mt = mp.tile([128, 288], f32)
                            m_ap = sub_ap(Mt[:], 0, [[16, 16], [1, 16]])
                            eng.tensor_tensor(out=m_ap, in0=t2_ap, in1=w_ap, op=mult)
                            eng.tensor_tensor(out=o_ap, in0=o_ap, in1=m_ap, op=add)
                # stores: m = mh*8 + mw + s*32, free (mh, i*16+j); one per half
                for s in range(2):
                    dst = AP(out_n[:].tensor, (s * 32 + mw) * (_C * 256),
                             [[256, _C], [8 * _C * 256, 4], [1, 256]])
                    nc.sync.dma_start(out=dst, in_=O4[s * 64:(s + 1) * 64, :])
    nc.compile()
    return nc


def _run(nc, x, w_all, **kwargs):
    from concourse.bass_utils import run_bass_kernel_spmd
    in_maps = [{"x_n": np.ascontiguousarray(x[n]), "w_all": w_all}
               for n in range(_N)]
    return run_bass_kernel_spmd(nc, in_maps, core_ids=list(range(_N)), **kwargs)


def _prepare(offset):
    bands, qh0, qh1, w_all = _plan(offset)
    nc = _build(bands, qh0, qh1)
    return nc, w_all


def kernel(x: np.ndarray, offset: np.ndarray) -> np.ndarray:
    x = np.asarray(x, dtype=np.float32)
    offset = np.asarray(offset, dtype=np.float32)
    nc, w_all = _prepare(offset)
    res = _run(nc, x, w_all)
    return np.stack([res.results[n]["out_n"] for n in range(_N)])


# revision 11
# speedup vs baseline: 1.5412x; 1.3303x over previous
"""Trainium2 Bass kernel for DeformablePatchSampler2d.

out[n, m, c, i, j] = bilinear_sample(x[n, c], row=RY[m, j], col=CX[m, i])

The sampling grid is batch/channel-invariant and known on the host from
`offset`, so all windows/weights are baked in at build time. Bilinear is
separable: stage 1 interpolates image rows (taps over j), stage 2 image
cols (taps over i).

v2 structure (per core, data-parallel over batch N=8):
  - 8 row-band DMA loads [128=(qh half, c), 20 rows x 126 cols] with both
    column-half origins folded into one AP (big segments, few descriptors)
  - per-patch 18x18 window gathers as ScalarE copies (no DMA descriptors)
  - stacks = patch pairs (mh, mw)+(mh+4, mw) -> both halves of the same
    band tile; f32 tensor_tensor on VectorE/GpSimdE with zero-stride
    broadcast weight APs
  - outputs merged 4 stacks per store [128, 4x256] -> one DMA per band
"""
import numpy as np

_P = 16
_NPH = _NPW = 8
_M = 64
_H = _W = 384
_C = 64
_N = 8
_QW = 126          # cols per column-half band
_RW = 20           # rows per row-band
_WPS = 96          # weight floats per stack: 2 stages * 3 taps * 16
_GPS_FRAC = 0.35   # fraction of stacks on gpsimd


def _precompute(offset: np.ndarray):
    """Window origins + 3-tap weights, f32 coord math mirroring the reference."""
    offset = offset.astype(np.float32)
    one, half = np.float32(1.0), np.float32(0.5)
    ch = np.linspace(0.0, float(_H), _NPH + 4).astype(np.float32)[2:-2]
    cw = np.linspace(0.0, float(_W), _NPW + 4).astype(np.float32)[2:-2]
    rel = np.arange(_P, dtype=np.float32) - np.float32(_P // 2)
    a = np.arange(_M) // _NPW
    b = np.arange(_M) % _NPW
    hc = ch[a][:, None] + rel[None, :]
    wcen = cw[b][:, None] + rel[None, :]
    gx = (np.float32(2.0) * hc / np.float32(_H - 1) - one) + offset[:, 0:1]
    gy = (np.float32(2.0) * wcen / np.float32(_W - 1) - one) + offset[:, 1:2]
    CX = (((gx + one) * np.float32(_W) - one) * half).astype(np.float64)  # (M,16) cols, dim i
    RY = (((gy + one) * np.float32(_H) - one) * half).astype(np.float64)  # (M,16) rows, dim j

    r0 = np.floor(RY[:, 0]).astype(np.int64)
    c0 = np.floor(CX[:, 0]).astype(np.int64)
    t_r = RY - (r0[:, None] + np.arange(_P)[None, :])
    t_c = CX - (c0[:, None] + np.arange(_P)[None, :])
    assert (t_r >= 0).all() and (t_r < 2).all()
    assert (t_c >= 0).all() and (t_c < 2).all()
    assert r0.min() >= 0 and (r0 + 17).max() <= _H - 1
    assert c0.min() >= 0 and (c0 + 17).max() <= _W - 1

    def taps(t):
        w0 = np.maximum(0.0, 1.0 - t)
        w2 = np.maximum(0.0, t - 1.0)
        return np.stack([w0, 1.0 - w0 - w2, w2], axis=-1).astype(np.float32)

    wr = taps(t_r)  # (M, 16, 3) applies to j (rows)
    wc = taps(t_c)  # (M, 16, 3) applies to i (cols)
    nt_r = np.where(np.abs(wr[:, :, 2]).max(axis=1) > 0, 3, 2)
    nt_c = np.where(np.abs(wc[:, :, 2]).max(axis=1) > 0, 3, 2)
    return r0, c0, wr, wc, nt_r, nt_c


def _plan(offset: np.ndarray):
    """Band layout, stack pairing, weight packing."""
    r0, c0, wr, wc, nt_r, nt_c = _precompute(offset)
    mh_of = np.arange(_M) // _NPW
    mw_of = np.arange(_M) % _NPW

    # row bands: one per mw; rows [band_r0, band_r0 + 20)
    band_r0 = np.array([r0[mw_of == mw].min() for mw in range(_NPW)])
    assert all((r0[m] + 17) - band_r0[mw_of[m]] <= _RW - 1 for m in range(_M))
    # column halves: qh0 = mh 0..3, qh1 = mh 4..7
    qh_start = np.array([c0[mh_of <= 3].min(), c0[mh_of >= 4].min()])
    assert (c0[mh_of <= 3] + 17 - qh_start[0]).max() <= _QW - 1
    assert (c0[mh_of >= 4] + 17 - qh_start[1]).max() <= _QW - 1
    assert qh_start[0] >= 0 and qh_start[1] + _QW <= _W

    w_all = np.zeros((128, 32 * _WPS), dtype=np.float32)
    bands = []
    for mw in range(_NPW):
        stacks = []
        for mh in range(4):
            ms = [mh * _NPW + mw, (mh + 4) * _NPW + mw]
            k = mw * 4 + mh
            rt = int(max(nt_r[ms[0]], nt_r[ms[1]]))
            ct = int(max(nt_c[ms[0]], nt_c[ms[1]]))
            for s, m in enumerate(ms):
                rows = slice(s * 64, (s + 1) * 64)
                base = k * _WPS
                for jk in range(3):
                    w_all[rows, base + jk * 16:base + jk * 16 + 16] = wr[m, :, jk][None, :]
                for ik in range(3):
                    w_all[rows, base + 48 + ik * 16:base + 48 + ik * 16 + 16] = wc[m, :, ik][None, :]
            stacks.append(dict(
                k=k, ms=ms, rt=rt, ct=ct,
                r_off=[int(r0[m] - band_r0[mw]) for m in ms],
                c_off=[int(c0[ms[s]] - qh_start[s]) for s in range(2)]))
        bands.append(dict(mw=mw, r0=int(band_r0[mw]), stacks=stacks))
    return bands, int(qh_start[0]), int(qh_start[1]), w_all


def _build(bands, qh0, qh1):
    import concourse.bacc as bacc
    import concourse.mybir as mybir
    from concourse.bass import AP
    from concourse.tile import TileContext

    f32 = mybir.dt.float32
    mult = mybir.AluOpType.mult
    add = mybir.AluOpType.add

    nc = bacc.Bacc("TRN2", target_bir_lowering=False)
    x_n = nc.dram_tensor("x_n", (_C, _H, _W), f32, kind="ExternalInput")
    w_d = nc.dram_tensor("w_all", (128, 32 * _WPS), f32, kind="ExternalInput")
    out_n = nc.dram_tensor("out_n", (_M, _C, _P, _P), f32, kind="ExternalOutput")

    def sub_ap(base_ap, extra_off, free_dims):
        return AP(base_ap.tensor, base_ap.offset + extra_off,
                  [list(base_ap.ap[0])] + [list(d) for d in free_dims])

    # which stacks run on gpsimd (spread evenly)
    n_gps = int(round(32 * _GPS_FRAC))
    gps_set = set(np.linspace(0, 31, n_gps, dtype=int).tolist())

    with TileContext(nc) as tc:
        with tc.tile_pool(name="wpool", bufs=1) as wpool, \
             tc.tile_pool(name="wpsp", bufs=1, space="PSUM") as wpsp, \
             tc.tile_pool(name="bpool", bufs=3) as bpool, \
             tc.tile_pool(name="apool", bufs=8) as apool, \
             tc.tile_pool(name="tpool", bufs=6) as tpool, \
             tc.tile_pool(name="mpool", bufs=6) as mpool, \
             tc.tile_pool(name="mpsp", bufs=2, space="PSUM") as mpsp, \
             tc.tile_pool(name="opool", bufs=3) as opool:
            W_sb = wpool.tile([128, 32 * _WPS], f32)
            nc.sync.dma_start(out=W_sb[:], in_=w_d[:])
            # PSUM copy of the weights: DVE ops read weights/temps via the
            # dedicated PSUM port, leaving the shared SBUF port pair to
            # GpSimd so both engines run truly concurrently.
            W_ps = wpsp.tile([128, 32 * _WPS], f32)
            nc.scalar.copy(out=W_ps[:], in_=W_sb[:])
            wb_s, wb_p = W_sb[:], W_ps[:]

            PREFETCH = 3  # bands in flight (= bpool bufs)

            def emit_load(band):
                # two DMAs, one per column half (disjoint partition halves ->
                # disjoint SDMA engine sets, so they overlap)
                B = bpool.tile([128, _RW * _QW], f32)
                for s, qs in enumerate((qh0, qh1)):
                    src = AP(x_n[:].tensor, band["r0"] * _W + qs,
                             [[_H * _W, _C], [_W, _RW], [1, _QW]])
                    nc.sync.dma_start(out=B[s * 64:(s + 1) * 64, :], in_=src)
                return B

            btiles = {b: emit_load(bands[b]) for b in range(PREFETCH)}
            for bi, band in enumerate(bands):
                mw = band["mw"]
                bb = btiles.pop(bi)[:]
                O4 = opool.tile([128, 4 * 256], f32)
                ob4 = O4[:]
                for st in band["stacks"]:
                    k = st["k"]
                    mh = k % 4
                    on_gps = k in gps_set
                    eng = nc.gpsimd if on_gps else nc.vector
                    wb = wb_s if on_gps else wb_p
                    mp = mpool if on_gps else mpsp
                    # gather 18x18 windows into the A stack (ScalarE copies)
                    A = apool.tile([128, 324], f32)
                    ab = A[:]
                    for s in range(2):
                        src_w = AP(bb.tensor,
                                   bb.offset + s * 64 * (_RW * _QW)
                                   + st["r_off"][s] * _QW + st["c_off"][s],
                                   [[_RW * _QW, 64], [_QW, 18], [1, 18]])
                        dst_w = AP(ab.tensor, ab.offset + s * 64 * 324,
                                   [[324, 64], [18, 18], [1, 18]])
                        nc.scalar.copy(out=dst_w, in_=src_w)
                    base = k * _WPS
                    qn = 16 + st["ct"]  # stage-2 only reads q in [0, 16+ct)
                    # stage 1: T[p, (j, q)] = sum_jk wr[jk, j] * A[p, (j+jk, q)]
                    T = tpool.tile([128, 288], f32)
                    tb = T[:]
                    t_ap = sub_ap(tb, 0, [[18, 16], [1, qn]])
                    for jk in range(st["rt"]):
                        a_ap = sub_ap(ab, jk * 18, [[18, 16], [1, qn]])
                        w_ap = sub_ap(wb, base + jk * 16, [[1, 16], [0, qn]])
                        if jk == 0:
                            eng.tensor_tensor(out=t_ap, in0=a_ap, in1=w_ap, op=mult)
                        else:
                            Mt = mp.tile([128, 288], f32)
                            m_ap = sub_ap(Mt[:], 0, [[18, 16], [1, qn]])
                            eng.tensor_tensor(out=m_ap, in0=a_ap, in1=w_ap, op=mult)
                            eng.tensor_tensor(out=t_ap, in0=t_ap, in1=m_ap, op=add)
                    # stage 2: O[p, (i, j)] = sum_ik wc[ik, i] * T[p, (j, i+ik)]
                    o_ap = sub_ap(ob4, mh * 256, [[16, 16], [1, 16]])
                    for ik in range(st["ct"]):
                        t2_ap = sub_ap(tb, ik, [[1, 16], [18, 16]])
                        w_ap = sub_ap(wb, base + 48 + ik * 16, [[1, 16], [0, 16]])
                        if ik == 0:
                            eng.tensor_tensor(out=o_ap, in0=t2_ap, in1=w_ap, op=mult)
                        else:
                            Mt = mp.tile([128, 288], f32)
                            m_ap = sub_ap(Mt[:], 0, [[16, 16], [1, 16]])
                            eng.tensor_tensor(out=m_ap, in0=t2_ap, in1=w_ap, op=mult)
                            eng.tensor_tensor(out=o_ap, in0=o_ap, in1=m_ap, op=add)
                # prefetch the next band's load BEFORE this band's stores so
                # the in-order Sync stream doesn't park loads behind stores
                if bi + PREFETCH < len(bands):
                    btiles[bi + PREFETCH] = emit_load(bands[bi + PREFETCH])
                # stores: m = mh*8 + mw + s*32, free (mh, i*16+j); one per half
                for s in range(2):
                    dst = AP(out_n[:].tensor, (s * 32 + mw) * (_C * 256),
                             [[256, _C], [8 * _C * 256, 4], [1, 256]])
                    nc.sync.dma_start(out=dst, in_=O4[s * 64:(s + 1) * 64, :])
    nc.compile()
    return nc


def _run(nc, x, w_all, **kwargs):
    from concourse.bass_utils import run_bass_kernel_spmd
    in_maps = [{"x_n": np.ascontiguousarray(x[n]), "w_all": w_all}
               for n in range(_N)]
    return run_bass_kernel_spmd(nc, in_maps, core_ids=list(range(_N)), **kwargs)


def _prepare(offset):
    bands, qh0, qh1, w_all = _plan(offset)
    nc = _build(bands, qh0, qh1)
    return nc, w_all


def kernel(x: np.ndarray, offset: np.ndarray) -> np.ndarray:
    x = np.asarray(x, dtype=np.float32)
    offset = np.asarray(offset, dtype=np.float32)
    nc, w_all = _prepare(offset)
    res = _run(nc, x, w_all)
    return np.stack([res.results[n]["out_n"] for n in range(_N)])
